# revision 18
# baseline (speedup 1.0000x reference)
"""Trainium2 Bass kernel for deformable conv2d (nn_DeformByDepthConv2d).

Strategy: data-parallel over batch (8 images -> 8 NeuronCores). Per core:
  1. Build a bilinear "difference table" in DRAM: for every padded pixel
     position r=(y,x), row = [V, Dx, Dy, Dxy] (64 ch each, bf16, 512B):
       V   = rgb[:, y, x]
       Dx  = rgb[:, y, x+1] - rgb[:, y, x]
       Dy  = rgb[:, y+1, x] - rgb[:, y, x]
       Dxy = rgb[:, y+1, x+1] - rgb[:, y+1, x] - rgb[:, y, x+1] + rgb[:, y, x]
     The bilinear sample is then exactly: V + fx*Dx + fy*Dy + fx*fy*Dxy
     with (fx, fy) the fractional sample coords -- so ONE dma_gather per
     (tap, pixel) fetches everything needed for a sample.
  2. Compute gather rows (int32) + poly weights (fx, fy, fx*fy) from
     `offsets` on-device (DVE). floor() is emulated robustly against either
     f32->int cast mode (HW rounds-to-nearest; CoreSim truncates), keeping
     (row, frac) consistent with the fp32 value they derive from.
  3. Gather: 9 taps x 32 pixel-groups indirect DMAs (SWDGE dynamic AP,
     128 rows x 512B per call) -> [128px, 32, 4, 64] per tap. One offset
     per partition is the only indirect-DMA shape this runtime executes
     correctly (the batched ext-isa dma_gather ucode is not shipped in
     this image), so the gather costs 288 x ~1.4us of GPSIMD time -- the
     kernel's dominant cost; everything else overlaps under it.
  4. DVE poly combine -> col[128px, 32, tap, 64ch] (bf16).
  5. PE transposes col -> colT[(tap,ch) 576, 4096px] (interleaved with the
     gather stream), then the conv is a [576,128]^T x [576,4096] matmul
     accumulated over 5 K-chunks in PSUM.
  6. ACT adds bias on PSUM->SBUF copy; DMA out fp32 [128, 4096].

Host side does layout-only prep (transposes/padding/replication of inputs);
all arithmetic (diffs, coords, weights, conv) runs on device.
"""

import os
import sys

sys.path.insert(0, "/opt/trn_rl_repo")

from contextlib import ExitStack

import numpy as np

import concourse.bass as bass
import concourse.tile as tile
from concourse import bacc, mybir
from concourse.bass_utils import run_bass_kernel_spmd
from concourse.masks import make_identity
from concourse.tile import add_dep_helper

F32 = mybir.dt.float32
BF16 = mybir.dt.bfloat16
I16 = mybir.dt.int16

B, CIN, COUT, KH, KW = 8, 64, 128, 3, 3
H = W = 64
K = KH * KW            # 9 taps
P = H * W              # 4096 pixels
PAD = 8                # table padding on each side
HP = WP = H + 2 * PAD  # 80
NROW = HP * WP         # 6400 table rows
NG = P // 128          # 32 pixel groups of 128
ELEM = 4 * CIN         # table row: V|Dx|Dy|Dxy x 64ch = 256 bf16 = 512B
KC = K * CIN           # 576 contraction rows
NK = 5                 # K chunks: 4x128 + 1x64
NCH = 8                # output N chunks of 512

# If the hardware float->int cast rounds-to-nearest instead of truncating,
# set to 0.5 (floor(v) == trunc(v - 0.5 + eps) for v > 0 non-half-integer).
CAST_BIAS = 0.0

TRACE = False
LAST_EXEC_NS = None
_PROG = None


def _build_program():
    nc = bacc.Bacc("TRN2", target_bir_lowering=False, debug=False, num_devices=8)

    # ---- DRAM tensors (per-core inputs; same program on all 8 cores) ----
    dt_in = lambda n, s, d=F32: nc.dram_tensor(n, s, d, kind="ExternalInput")
    rv = dt_in("rv", [NROW, CIN])       # V layout     [6400, 64]
    rx = dt_in("rx", [NROW, CIN])       # V(x+1)
    ry = dt_in("ry", [NROW, CIN])       # V(y+1)
    rxy = dt_in("rxy", [NROW, CIN])     # V(x+1,y+1)
    offn = dt_in("offn", [P, 2 * K])    # offsets, pixel-major [4096, 18]
    cyn = dt_in("cyn", [128, NG, K])    # iy+ky+15 (natural layout)
    cxn = dt_in("cxn", [128, NG, K])    # ix+kx+15
    w2t = dt_in("w2t", [KC, COUT])      # weight[(k,c), o]
    biasv = dt_in("biasv", [COUT, 1])
    tblD = nc.dram_tensor("tblD", [NROW, ELEM], BF16, kind="Internal")
    outD = nc.dram_tensor("outD", [COUT, P], F32, kind="ExternalOutput")

    with tile.TileContext(nc) as tc, ExitStack() as ctx:
        consts = ctx.enter_context(tc.tile_pool(name="consts", bufs=1))
        ident = consts.tile([128, 128], BF16)
        make_identity(nc, ident[:])
        bias_sb = consts.tile([COUT, 1], F32)
        nc.sync.dma_start(bias_sb[:], biasv.ap())

        # weight chunks (SWDGE cast f32->bf16 on load)
        wts = []
        for j in range(NK):
            cs = 128 if j < 4 else 64
            wt = consts.tile([cs, COUT], BF16, tag=f"wt{j}", name=f"wt{j}")
            nc.gpsimd.dma_start(wt[:], w2t.ap()[j * 128:j * 128 + cs, :])
            wts.append(wt)

        # ---------------- phase 1: build the diff table ----------------
        with tc.tile_pool(name="tblsrc", bufs=1) as tp:
            r3 = lambda t: t.ap().rearrange("(g p) c -> p g c", p=128)
            v_sb = tp.tile([128, NROW // 128, CIN], F32, tag="v")
            x_sb = tp.tile([128, NROW // 128, CIN], F32, tag="x")
            y_sb = tp.tile([128, NROW // 128, CIN], F32, tag="y")
            xy_sb = tp.tile([128, NROW // 128, CIN], F32, tag="xy")
            nc.sync.dma_start(v_sb[:], r3(rv))
            nc.sync.dma_start(x_sb[:], r3(rx))
            nc.sync.dma_start(y_sb[:], r3(ry))
            nc.sync.dma_start(xy_sb[:], r3(rxy))

            tbl = tp.tile([128, NROW // 128, 4, CIN], BF16, tag="tbl")
            t3 = tp.tile([128, NROW // 128, CIN], F32, tag="t3")
            t4 = tp.tile([128, NROW // 128, CIN], F32, tag="t4")
            nc.vector.tensor_copy(tbl[:, :, 0, :], v_sb[:])
            nc.vector.tensor_sub(tbl[:, :, 1, :], x_sb[:], v_sb[:])
            nc.vector.tensor_sub(t4[:], y_sb[:], v_sb[:])
            nc.vector.tensor_copy(tbl[:, :, 2, :], t4[:])
            nc.vector.tensor_sub(t3[:], xy_sb[:], x_sb[:])
            nc.vector.tensor_sub(tbl[:, :, 3, :], t3[:], t4[:])

            tbl_store = nc.sync.dma_start(
                tblD.ap().rearrange("(g p) e -> p g e", p=128),
                tbl[:].rearrange("p g v c -> p g (v c)"),
            )

        prep = ctx.enter_context(tc.tile_pool(name="prep", bufs=1))
        idxn = prep.tile([128, NG, K], mybir.dt.int32, tag="idxn")
        wpoly = prep.tile([128, NG, K, 3], BF16, tag="wpoly")

        with tc.tile_pool(name="prept", bufs=1) as pt:
            # ---- phase 2: sample coords -> gather rows + poly weights ----
            offn_sb = pt.tile([128, NG, 2 * K], F32, tag="offn")
            nc.sync.dma_start(
                offn_sb[:], offn.ap().rearrange("(g p) c -> p g c", p=128)
            )
            cyn_sb = pt.tile([128, NG, K], F32, tag="cyn")
            cxn_sb = pt.tile([128, NG, K], F32, tag="cxn")
            nc.sync.dma_start(cyn_sb[:], cyn.ap())
            nc.sync.dma_start(cxn_sb[:], cxn.ap())

            yn = pt.tile([128, NG, K], F32, tag="yn")
            xn = pt.tile([128, NG, K], F32, tag="xn")
            yc = pt.tile([128, NG, K], F32, tag="yc")
            xc = pt.tile([128, NG, K], F32, tag="xc")
            yni = pt.tile([128, NG, K], mybir.dt.int32, tag="yni")
            xni = pt.tile([128, NG, K], mybir.dt.int32, tag="xni")
            ynf = pt.tile([128, NG, K], F32, tag="ynf")
            xnf = pt.tile([128, NG, K], F32, tag="xnf")
            idx0 = pt.tile([128, NG, K], mybir.dt.int32, tag="idx0")
            nc.vector.tensor_add(yn[:], offn_sb[:, :, 0:2 * K:2], cyn_sb[:])
            nc.vector.tensor_add(xn[:], offn_sb[:, :, 1:2 * K:2], cxn_sb[:])
            # +16-shifted coords; clamp to [8, 86.99]. floor() robust to the
            # cast mode (HW rounds-to-nearest, CoreSim truncates): cast, cast
            # back, subtract 1 wherever the cast went up.
            nc.vector.tensor_scalar(
                yc[:], yn[:], 8.0, 86.99,
                mybir.AluOpType.max, mybir.AluOpType.min,
            )
            nc.vector.tensor_scalar(
                xc[:], xn[:], 8.0, 86.99,
                mybir.AluOpType.max, mybir.AluOpType.min,
            )
            ym = pt.tile([128, NG, K], F32, tag="ym")
            xm = pt.tile([128, NG, K], F32, tag="xm")
            nc.vector.tensor_copy(yni[:], yc[:])
            nc.vector.tensor_copy(xni[:], xc[:])
            nc.vector.tensor_copy(ynf[:], yni[:])
            nc.vector.tensor_copy(xnf[:], xni[:])
            nc.vector.tensor_tensor(ym[:], ynf[:], yc[:], mybir.AluOpType.is_gt)
            nc.vector.tensor_tensor(xm[:], xnf[:], xc[:], mybir.AluOpType.is_gt)
            ynf2 = pt.tile([128, NG, K], F32, tag="ynf2")
            xnf2 = pt.tile([128, NG, K], F32, tag="xnf2")
            nc.vector.tensor_sub(ynf2[:], ynf[:], ym[:])
            nc.vector.tensor_sub(xnf2[:], xnf[:], xm[:])
            nc.vector.tensor_copy(yni[:], ynf2[:])
            nc.vector.tensor_copy(xni[:], xnf2[:])
            nc.vector.tensor_sub(wpoly[:, :, :, 1], yn[:], ynf2[:])   # fy
            nc.vector.tensor_sub(wpoly[:, :, :, 0], xn[:], xnf2[:])   # fx
            nc.vector.tensor_mul(
                wpoly[:, :, :, 2], wpoly[:, :, :, 0], wpoly[:, :, :, 1]
            )  # fx*fy
            # table row = (y0+16-8)*80 + (x0+16-8) = 80*yni + xni - 648
            nc.vector.scalar_tensor_tensor(
                idx0[:], yni[:], float(WP), xni[:],
                mybir.AluOpType.mult, mybir.AluOpType.add,
            )
            nc.vector.tensor_scalar(
                idxn[:], idx0[:], -648.0, None, mybir.AluOpType.add,
            )

        # ---- phase 3: gathers + poly combine -> col; transposes interleave ----
        colp = ctx.enter_context(tc.tile_pool(name="colp", bufs=1))
        col = colp.tile([128, NG, K, CIN], BF16)
        gpool = ctx.enter_context(tc.tile_pool(name="gath", bufs=2))
        ppool = ctx.enter_context(tc.tile_pool(name="prod", bufs=2))
        apool = ctx.enter_context(tc.tile_pool(name="acc", bufs=2))
        ctp = ctx.enter_context(tc.tile_pool(name="colT", bufs=1))
        cts = []
        for j in range(NK):
            cs = 128 if j < 4 else 64
            cts.append(ctp.tile([cs, P], BF16, tag=f"ct{j}", name=f"ct{j}"))
        pst = ctx.enter_context(tc.tile_pool(name="pst", bufs=4, space="PSUM"))

        def transpose_chunk(j):
            # colT chunk j (k,c)-rows [128j, 128j+cs) <- needs taps 2j..2j+2
            cs = 128 if j < 4 else 64
            for g_i in range(NG):
                src = bass.AP(
                    col[:].tensor,
                    col[:].offset + g_i * (K * CIN) + j * 128,
                    [list(col[:].ap[0]), [1, cs]],
                )
                ptile = pst.tile([cs, 128], BF16, tag="pt", name="pt")
                nc.tensor.transpose(ptile[:], src, ident[:])
                nc.scalar.copy(cts[j][:, g_i * 128:(g_i + 1) * 128], ptile[:])

        for k in range(K):
            g = gpool.tile([128, NG, ELEM], BF16, tag="g")
            for g_i in range(NG):
                gi = nc.gpsimd.indirect_dma_start(
                    out=g[:, g_i, :],
                    out_offset=None,
                    in_=tblD.ap(),
                    in_offset=bass.IndirectOffsetOnAxis(
                        ap=idxn[:, g_i, k:k + 1], axis=0
                    ),
                )
                add_dep_helper(
                    gi.ins, tbl_store.ins, reason="gather reads diff table"
                )

            gv = g[:].rearrange("p n (v c) -> p n v c", v=4)
            wk = wpoly[:, :, k, :]
            wkb = bass.AP(wk.tensor, wk.offset, list(wk.ap) + [[0, CIN]])
            pr = ppool.tile([128, NG, 3, CIN], BF16, tag="pr")
            nc.vector.tensor_mul(pr[:], gv[:, :, 1:4, :], wkb)
            a1 = apool.tile([128, NG, CIN], BF16, tag="a1")
            a2 = apool.tile([128, NG, CIN], BF16, tag="a2")
            nc.vector.tensor_add(a1[:], gv[:, :, 0, :], pr[:, :, 0, :])
            nc.vector.tensor_add(a2[:], a1[:], pr[:, :, 1, :])
            nc.vector.tensor_add(col[:, :, k, :], a2[:], pr[:, :, 2, :])
            # kick off any colT chunks fully covered by taps <= k
            # (chunk j covers rows [128j, 128j+cs) = taps {2j, 2j+1}; chunk 4 = tap 8)
            for j in range(NK):
                if k == min(2 * j + 1, K - 1) and (j < NK - 1 or k == K - 1):
                    transpose_chunk(j)

        # ------------- phase 5: matmul + bias + store -------------
        psm = ctx.enter_context(tc.tile_pool(name="psm", bufs=4, space="PSUM"))
        obp = ctx.enter_context(tc.tile_pool(name="obp", bufs=2))
        for n in range(NCH):
            pm = psm.tile([COUT, P // NCH], F32, tag="pm")
            for j in range(NK):
                nc.tensor.matmul(
                    pm[:],
                    wts[j][:],
                    cts[j][:, n * (P // NCH):(n + 1) * (P // NCH)],
                    start=(j == 0),
                    stop=(j == NK - 1),
                )
            ob = obp.tile([COUT, P // NCH], F32, tag="ob")
            nc.scalar.activation(
                ob[:], pm[:], mybir.ActivationFunctionType.Identity,
                bias=bias_sb[:], scale=1.0,
            )
            nc.sync.dma_start(
                outD.ap()[:, n * (P // NCH):(n + 1) * (P // NCH)], ob[:]
            )

    nc.compile()
    return nc


def _host_prep(rgb, offsets, weight, bias):
    """Layout-only host prep -> per-core input maps."""
    rgb = np.ascontiguousarray(np.asarray(rgb, dtype=np.float32))
    offsets = np.ascontiguousarray(np.asarray(offsets, dtype=np.float32))
    weight = np.asarray(weight, dtype=np.float32)
    bias = np.asarray(bias, dtype=np.float32)

    w2t = np.ascontiguousarray(
        weight.transpose(2, 3, 1, 0).reshape(KC, COUT)
    )
    biasv = np.ascontiguousarray(bias.reshape(COUT, 1))

    ky = (np.arange(K) // 3).astype(np.float32)
    kx = (np.arange(K) % 3).astype(np.float32)
    pix = np.arange(P)
    iy = (pix // W).astype(np.float32)
    ix = (pix % W).astype(np.float32)

    # natural layout [128, 32, 9]: pixel p=(g*128+part)
    def nat(base, kk):
        c = base[:, None] + kk[None, :]          # [4096, 9]
        return np.ascontiguousarray(
            c.reshape(NG, 128, K).transpose(1, 0, 2)
        )

    cyn = nat(iy + 15.0, ky)
    cxn = nat(ix + 15.0, kx)

    in_maps = []
    for b in range(B):
        canvas = np.zeros((CIN, H + 18, W + 18), np.float32)
        canvas[:, PAD:PAD + H, PAD:PAD + W] = rgb[b]
        mk = lambda sy, sx: np.ascontiguousarray(
            canvas[:, sy:sy + HP, sx:sx + WP].transpose(1, 2, 0).reshape(NROW, CIN)
        )
        offs = offsets[b].reshape(2 * K, P)
        in_maps.append({
            "rv": mk(0, 0), "rx": mk(0, 1), "ry": mk(1, 0), "rxy": mk(1, 1),
            "offn": np.ascontiguousarray(offs.T),          # [4096, 18]
            "cyn": cyn, "cxn": cxn,
            "w2t": w2t, "biasv": biasv,
        })
    return in_maps


def _axon_reset():
    try:
        import ctypes

        import jax

        jax.devices()
        lib = ctypes.CDLL("/opt/axon/libaxon_pjrt.so")
        lib.axon_reset.restype = ctypes.c_int64
        lib.axon_reset()
    except Exception:
        pass


def kernel(rgb, offsets, weight, bias):
    global _PROG, LAST_EXEC_NS
    if _PROG is None:
        _PROG = _build_program()
    in_maps = _host_prep(rgb, offsets, weight, bias)
    try:
        res = run_bass_kernel_spmd(
            _PROG, in_maps, core_ids=list(range(B)), trace=TRACE
        )
    except Exception:
        # a previous crashed run can leave the device wedged; reset + retry
        _axon_reset()
        res = run_bass_kernel_spmd(
            _PROG, in_maps, core_ids=list(range(B)), trace=TRACE
        )
    LAST_EXEC_NS = res.exec_time_ns
    out = np.stack([res.results[b]["outD"] for b in range(B)])
    return out.reshape(B, COUT, H, W).astype(np.float32)
